# revision 1
# baseline (speedup 1.0000x reference)
"""CTC loss (focal-reweighted) Trainium2 Bass kernel.

Strategy: pure data parallel over batch (128 examples -> 8 cores x 16).
Per core:
  - stream x tiles of [8 examples x 16 timesteps, C] (host-permuted rows so
    each tile is one contiguous DMA); exp on ACT with accum_out -> softmax
    denominators Z[b,t]
  - ap_gather (GPSIMD) pulls per-(b,t) emission values out of the exp'd tile
    directly in extended-label order (51 states: blanks interleaved, blank
    value replicated by the gather); every 16-partition group is one
    example's 16 timesteps, so groups share their index list
  - gathered values reshuffle SBUF->SBUF into per-t-block chunks so the CTC
    forward DP (prob space, renorm every 8 steps, 4 full-width vector ops
    per step) pipelines behind the streaming
  - loss = -(log v + sum log S - sum log Z), focal reweight, per-example out
Host: shards inputs, computes label-derived index/mask tensors, means the
128 per-example losses.
"""

import numpy as np

import concourse.bass as bass
import concourse.bacc as bacc
import concourse.tile as tile
from concourse import mybir
from concourse import bass_utils

B, T, C, L = 128, 160, 6625, 25
NCORES = 8
BL = B // NCORES          # 16 examples per core
S = 2 * L + 1             # 51 extended states
NI = 64                   # ap_gather num_idxs (S padded to a multiple of 16)
TBJ = 10                  # t-blocks of 16 timesteps
NT = 2 * TBJ              # 20 streaming tiles of [128, C]
RENORM = 8
NREN = 20                 # renorms: t = 8,16,...,152 and t = 159

F32 = mybir.dt.float32
I16 = mybir.dt.int16
U32 = mybir.dt.uint32
LN2 = 0.6931471805599453


def _build_kernel():
    nc = bacc.Bacc("TRN2", target_bir_lowering=False, debug=False)
    x = nc.dram_tensor("x", [BL * T, C], F32, kind="ExternalInput").ap()
    gidx = nc.dram_tensor("gidx", [128, NT * 4], I16, kind="ExternalInput").ap()
    m51 = nc.dram_tensor("m51", [BL, S], F32, kind="ExternalInput").ap()
    sel = nc.dram_tensor("sel", [BL, S + 2], F32, kind="ExternalInput").ap()
    loss16 = nc.dram_tensor("loss16", [BL, 1], F32, kind="ExternalOutput").ap()

    with tile.TileContext(nc) as tc:
        with (
            tc.tile_pool(name="xio", bufs=5) as xio,
            tc.tile_pool(name="small", bufs=1) as small,
            tc.tile_pool(name="dram", bufs=1, space="DRAM") as dpool,
        ):
            gidx_sb = small.tile([128, NT * 4], I16)
            nc.sync.dma_start(out=gidx_sb[:, :], in_=gidx[:, :])
            m51_sb = small.tile([BL, S], F32)
            nc.sync.dma_start(out=m51_sb[:, :], in_=m51[:, :])
            sel_sb = small.tile([BL, S + 2], F32)
            nc.sync.dma_start(out=sel_sb[:, :], in_=sel[:, :])

            # ---- streaming: tile i = 2j+o holds examples [8o, 8o+8) x
            # timesteps [16j, 16j+16); partition p = b_loc*16 + t_fine ----
            Z = small.tile([128, NT], F32)
            xv = x.rearrange("(n p) c -> n p c", p=128)
            e51c = []
            for j in range(TBJ):
                ec = small.tile([BL, 16 * S], F32, tag=f"e51c{j}")
                ecv = ec[:, :].rearrange("b (t s) -> b t s", s=S)
                for o in range(2):
                    i = 2 * j + o
                    xt = xio.tile([128, C], F32)
                    nc.sync.dma_start(out=xt[:, :], in_=xv[i, :, :])
                    nc.scalar.activation(out=xt[:, :], in_=xt[:, :],
                                         func=mybir.ActivationFunctionType.Exp,
                                         accum_out=Z[:, i:i + 1])
                    ga = small.tile([128, NI], F32, tag=f"ga{i}")
                    nc.gpsimd.ap_gather(
                        out_ap=ga[:, :].rearrange("p (n d) -> p n d", d=1),
                        in_ap=xt[:, :].rearrange("p (n d) -> p n d", d=1),
                        idxs_ap=gidx_sb[:, i * 4:(i + 1) * 4],
                        channels=128, num_elems=C, d=1, num_idxs=NI,
                    )
                    # SBUF->SBUF partition reshuffle straight into the DP
                    # chunk, issued from GPSIMD (SWDGE): it directly follows
                    # the gather on the same engine, so its wait never stalls
                    # the x-load ring the way a HWDGE-sequencer wait would
                    nc.gpsimd.dma_start(out=ecv[8 * o:8 * o + 8, :, :],
                                        in_=ga[:, 0:S])
                e51c.append(ec)

            # ---- Z -> per-example sum of log Z via DRAM bounce ----
            zd = dpool.tile([BL * T], F32)
            nc.sync.dma_start(out=zd.rearrange("(f p) -> p f", p=128), in_=Z[:, :])
            Zt = small.tile([BL, T], F32)
            zd0 = zd[:]
            for o in range(2):
                zsrc = bass.AP(tensor=zd0.tensor, offset=zd0.offset + 128 * o,
                               ap=[[16, 8], [256, TBJ], [1, 16]])
                nc.scalar.dma_start(out=Zt[8 * o:8 * o + 8, :], in_=zsrc)
            nc.scalar.activation(out=Zt[:, :], in_=Zt[:, :],
                                 func=mybir.ActivationFunctionType.Ln)
            slZ = small.tile([BL, 1], F32)
            nc.vector.reduce_sum(out=slZ[:, :], in_=Zt[:, :],
                                 axis=mybir.AxisListType.X)

            # ---- CTC forward DP in rescaled prob space ----
            # alpha buffers have 2 guard columns (always 0); state s at
            # col s+2, so cur[:, 0:S] reads alpha[s-2] (guards give 0)
            A = small.tile([BL, S + 2], F32)
            Bb = small.tile([BL, S + 2], F32)
            w51 = small.tile([BL, S], F32)
            Sbuf = small.tile([BL, NREN], F32)
            rec = small.tile([BL, 1], F32)
            nc.vector.memset(A[:, :], 0.0)
            nc.vector.memset(Bb[:, :], 0.0)
            # init: alpha0[0] = e(t=0, blank), alpha0[1] = e(t=0, label0)
            # (on ACT: the DVE copy would need two sync waits at this join)
            nc.scalar.copy(out=A[:, 2:4], in_=e51c[0][:, 0:2])

            cur, nxt = A, Bb
            k = 0  # renorm slot
            for t in range(1, T):
                et = e51c[t // 16][:, (t % 16) * S:(t % 16 + 1) * S]
                # nxt[s] = (cur[s] + cur[s-1] + allow_skip[s]*cur[s-2]) * e_t[s]
                nc.vector.tensor_add(out=nxt[:, 2:S + 2], in0=cur[:, 2:S + 2],
                                     in1=cur[:, 1:S + 1])
                nc.vector.tensor_mul(out=w51[:, :], in0=cur[:, 0:S],
                                     in1=m51_sb[:, :])
                nc.vector.tensor_add(out=nxt[:, 2:S + 2],
                                     in0=nxt[:, 2:S + 2], in1=w51[:, :])
                nc.vector.tensor_mul(out=nxt[:, 2:S + 2],
                                     in0=nxt[:, 2:S + 2], in1=et)
                cur, nxt = nxt, cur
                if (t % RENORM == 0 and t <= 152) or t == T - 1:
                    nc.vector.reduce_sum(out=Sbuf[:, k:k + 1], in_=cur[:, 2:S + 2],
                                         axis=mybir.AxisListType.X)
                    nc.vector.reciprocal(out=rec[:, :], in_=Sbuf[:, k:k + 1])
                    nc.vector.tensor_scalar_mul(out=cur[:, 2:S + 2],
                                                in0=cur[:, 2:S + 2],
                                                scalar1=rec[:, :])
                    k += 1
            assert k == NREN

            # ---- readout ----
            # v = alpha[2*len] + alpha[2*len-1] via host-built selection mask
            nc.vector.tensor_mul(out=nxt[:, :], in0=cur[:, :], in1=sel_sb[:, :])
            v = small.tile([BL, 1], F32)
            nc.vector.reduce_sum(out=v[:, :], in_=nxt[:, :],
                                 axis=mybir.AxisListType.X)
            # log v with v possibly ~e^-80: the ACT Ln table is only accurate
            # for inputs in ~e^[-40, 40], so split v into IEEE exponent and
            # mantissa and only table-Ln the mantissa (in [1, 2))
            ebits = small.tile([BL, 1], U32)
            mbits = small.tile([BL, 1], U32)
            exf = small.tile([BL, 1], F32)
            nc.vector.tensor_scalar(out=ebits[:, :], in0=v[:, :].bitcast(U32),
                                    scalar1=23, scalar2=None,
                                    op0=mybir.AluOpType.logical_shift_right)
            nc.vector.tensor_copy(out=exf[:, :], in_=ebits[:, :])
            nc.vector.tensor_scalar(out=mbits[:, :], in0=v[:, :].bitcast(U32),
                                    scalar1=0x7FFFFF, scalar2=0x3F800000,
                                    op0=mybir.AluOpType.bitwise_and,
                                    op1=mybir.AluOpType.bitwise_or)
            nc.scalar.activation(out=v[:, :], in_=mbits[:, :].bitcast(F32),
                                 func=mybir.ActivationFunctionType.Ln)
            # v = ln(mantissa) + (exponent - 127) * ln2
            nc.vector.tensor_scalar(out=exf[:, :], in0=exf[:, :],
                                    scalar1=LN2, scalar2=-127.0 * LN2,
                                    op0=mybir.AluOpType.mult,
                                    op1=mybir.AluOpType.add)
            nc.vector.tensor_add(out=v[:, :], in0=v[:, :], in1=exf[:, :])
            # sum log S
            nc.scalar.activation(out=Sbuf[:, :], in_=Sbuf[:, :],
                                 func=mybir.ActivationFunctionType.Ln)
            slS = small.tile([BL, 1], F32)
            nc.vector.reduce_sum(out=slS[:, :], in_=Sbuf[:, :],
                                 axis=mybir.AxisListType.X)
            # loss = slZ - (log v + slS)
            lt = small.tile([BL, 1], F32)
            nc.vector.tensor_add(out=lt[:, :], in0=v[:, :], in1=slS[:, :])
            nc.vector.tensor_tensor(out=lt[:, :], in0=slZ[:, :], in1=lt[:, :],
                                    op=mybir.AluOpType.subtract)
            # focal: w = (exp(-loss) - 1)^2 ; out = loss * w
            # (clamp the exp input: the ACT Exp table degrades far out of range)
            em = small.tile([BL, 1], F32)
            nc.vector.tensor_scalar_min(out=em[:, :], in0=lt[:, :], scalar1=80.0)
            nc.scalar.activation(out=em[:, :], in_=em[:, :],
                                 func=mybir.ActivationFunctionType.Exp,
                                 scale=-1.0)
            nc.vector.tensor_scalar_add(out=em[:, :], in0=em[:, :], scalar1=-1.0)
            nc.vector.tensor_mul(out=em[:, :], in0=em[:, :], in1=em[:, :])
            nc.vector.tensor_mul(out=lt[:, :], in0=lt[:, :], in1=em[:, :])
            nc.scalar.dma_start(out=loss16[:, :], in_=lt[:, :])

    nc.compile()
    return nc


def _prep_core(predicts, labels, label_lengths, b0):
    """Host-side shard prep for examples [b0, b0+BL)."""
    # permute rows to (t_block, example, t_fine) so streaming tile i = 2j+o
    # holds examples [8o, 8o+8) x timesteps [16j, 16j+16) as 128 contiguous
    # rows (partition p = b_loc*16 + t_fine)
    xs = np.asarray(predicts[b0:b0 + BL], dtype=np.float32)
    xs = np.ascontiguousarray(
        xs.reshape(BL, TBJ, 16, C).transpose(1, 0, 2, 3).reshape(BL * T, C))
    lab = labels[b0:b0 + BL].astype(np.int64)            # [BL, L]
    lens = label_lengths[b0:b0 + BL].astype(np.int64)    # [BL]
    # extended-label class ids per state: even s -> blank 0, odd s -> label
    ext = np.zeros((BL, NI), np.int64)
    ext[:, 1:S:2] = lab
    # ap_gather index tiles: streaming tile i, partition p -> example
    # 8*(i%2) + p//16; slot s holds state-class ext[b][s*16 + p%16]
    i_idx = np.arange(NT)[:, None, None]
    p_idx = np.arange(128)[None, :, None]
    s_idx = np.arange(4)[None, None, :]
    b_of = 8 * (i_idx % 2) + p_idx // 16
    k_of = s_idx * 16 + (p_idx % 16)
    gidx = ext[b_of, k_of]                               # [NT, 128, 4]
    gidx = gidx.transpose(1, 0, 2).reshape(128, NT * 4).astype(np.int16)
    # skip-allowed mask in extended-state space (odd states only, no repeat)
    m51 = np.zeros((BL, S), np.float32)
    m51[:, 3::2] = (lab[:, 1:] != lab[:, :-1]).astype(np.float32)
    sel = np.zeros((BL, S + 2), np.float32)
    rows = np.arange(BL)
    sel[rows, 2 * lens + 2] = 1.0
    sel[rows, 2 * lens + 1] = 1.0
    return {"x": xs, "gidx": gidx, "m51": m51, "sel": sel}


_NC_CACHE = []


def kernel(predicts, labels, label_lengths):
    predicts = np.asarray(predicts)
    labels = np.asarray(labels)
    label_lengths = np.asarray(label_lengths)
    if not _NC_CACHE:
        _NC_CACHE.append(_build_kernel())
    nc = _NC_CACHE[0]
    in_maps = [
        _prep_core(predicts, labels, label_lengths, k * BL) for k in range(NCORES)
    ]
    res = bass_utils.run_bass_kernel_spmd(nc, in_maps, core_ids=list(range(NCORES)))
    losses = np.concatenate([r["loss16"].reshape(BL) for r in res.results])
    return np.float32(np.mean(losses.astype(np.float64)))



# revision 3
# speedup vs baseline: 1.5093x; 1.5093x over previous
"""CTC loss (focal-reweighted) Trainium2 Bass kernel.

Strategy: pure data parallel over batch (128 examples -> 8 cores x 16).
Per core:
  - stream x tiles of [8 examples x 16 timesteps, C] in bf16 (host-cast;
    halves the dominant HBM traffic); exp on ACT (bf16 in -> f32 out tile)
    with accum_out -> softmax denominators Z[b,t]
  - ap_gather (GPSIMD) pulls per-(b,t) emission values out of the exp'd
    tile in extended-label order TWICE: slots 0..50 are e[s], slots
    52..102 are em[s] = e[s]*allow_skip[s] (mask folded into the gather by
    pointing disallowed slots at a zeroed column past C) - this removes
    the mask multiply from the DP chain
  - gathered values reshuffle SBUF->SBUF into per-t-block chunks (GPSIMD
    SWDGE queue) so the CTC forward DP pipelines behind the streaming
  - DP in rescaled prob space: 4 DVE ops/step; renorm every 8 steps is
    folded in (sum via scalar_tensor_tensor accum_out on the step's final
    add, 1/S applied by scalar_tensor_tensor on the NEXT step's two
    product terms) so it only costs one reciprocal on the serial chain
  - outputs alpha_T, the 19 renorm sums and Z raw; host does all the
    log/focal/mean scalar math in float64
"""

import numpy as np
import ml_dtypes

import concourse.bass as bass
import concourse.bacc as bacc
import concourse.tile as tile
from concourse import mybir
from concourse import bass_utils

B, T, C, L = 128, 160, 6625, 25
NCORES = 8
BL = B // NCORES          # 16 examples per core
S = 2 * L + 1             # 51 extended states
NI = 112                  # ap_gather num_idxs (7 idx cols x 16 partitions)
G = 104                   # chunk stride: e at 0..50, em at 52..102
TBJ = 10                  # t-blocks of 16 timesteps
NT = 2 * TBJ              # 20 streaming tiles of [128, C]
CP = C + 8                # exp'd tile width; cols C.. stay 0 (em mask)
RENORM = 8
NREN = 19                 # renorms at t = 8,16,...,152
NXIN = 4                  # bf16 input tile ring
NXOUT = 3                 # f32 exp'd tile ring
NGA = 4                   # gather output ring

F32 = mybir.dt.float32
BF16 = mybir.dt.bfloat16
I16 = mybir.dt.int16


def _build_kernel():
    nc = bacc.Bacc("TRN2", target_bir_lowering=False, debug=False)
    x = nc.dram_tensor("x", [BL * T, C], BF16, kind="ExternalInput").ap()
    gidx = nc.dram_tensor("gidx", [128, NT * 7], I16, kind="ExternalInput").ap()
    alpha_out = nc.dram_tensor("alpha_out", [BL, S + 2], F32,
                               kind="ExternalOutput").ap()
    sb_out = nc.dram_tensor("sb_out", [BL, NREN], F32,
                            kind="ExternalOutput").ap()
    z_out = nc.dram_tensor("z_out", [128, NT], F32, kind="ExternalOutput").ap()

    MUL = mybir.AluOpType.mult
    ADD = mybir.AluOpType.add

    with tile.TileContext(nc) as tc:
        with tc.tile_pool(name="small", bufs=1) as small:
            gidx_sb = small.tile([128, NT * 7], I16)
            nc.scalar.dma_start(out=gidx_sb[:, :], in_=gidx[:, :])

            xin = [small.tile([128, C], BF16, name=f"xin{n}")
                   for n in range(NXIN)]
            xout = [small.tile([128, CP], F32, name=f"xout{n}")
                    for n in range(NXOUT)]
            ga = [small.tile([128, NI], F32, name=f"ga{n}") for n in range(NGA)]
            for n in range(NXOUT):
                # zero guard columns: em-gather indices for masked slots
                # point at col C, so these must stay 0 (exp only writes :C)
                nc.vector.memset(xout[n][:, C:CP], 0.0)

            Z = small.tile([128, NT], F32)
            e51c = [small.tile([BL, 16 * G], F32, name=f"e51c{j}")
                    for j in range(TBJ)]

            # ---- streaming: tile i = 2j+o holds examples [8o, 8o+8) x
            # timesteps [16j, 16j+16); partition p = b_loc*16 + t_fine ----
            xv = x.rearrange("(n p) c -> n p c", p=128)
            for i in range(NT):
                xt = xin[i % NXIN]
                nc.sync.dma_start(out=xt[:, :], in_=xv[i, :, :])
                xo = xout[i % NXOUT]
                nc.scalar.activation(out=xo[:, 0:C], in_=xt[:, :],
                                     func=mybir.ActivationFunctionType.Exp,
                                     accum_out=Z[:, i:i + 1])
                g = ga[i % NGA]
                nc.gpsimd.ap_gather(
                    out_ap=g[:, :].rearrange("p (n d) -> p n d", d=1),
                    in_ap=xo[:, :].rearrange("p (n d) -> p n d", d=1),
                    idxs_ap=gidx_sb[:, i * 7:(i + 1) * 7],
                    channels=128, num_elems=CP, d=1, num_idxs=NI,
                )
                # SBUF->SBUF partition reshuffle straight into the DP chunk
                # (SWDGE so it doesn't FIFO behind x-loads on the sync ring)
                j, o = i // 2, i % 2
                ecv = e51c[j][:, :].rearrange("b (t g) -> b t g", g=G)
                nc.gpsimd.dma_start(out=ecv[8 * o:8 * o + 8, :, :],
                                    in_=g[:, 0:G])

            # ---- CTC forward DP in rescaled prob space ----
            # alpha buffers have 2 guard columns (always 0); state s at
            # col s+2, so cur[:, 0:S] reads alpha[s-2] (guards give 0)
            A = small.tile([BL, S + 2], F32)
            Bb = small.tile([BL, S + 2], F32)
            u = small.tile([BL, S], F32)
            w = small.tile([BL, S], F32)
            Sb = small.tile([BL, NREN], F32)
            rec = small.tile([BL, 1], F32)
            nc.vector.memset(A[:, :], 0.0)
            nc.vector.memset(Bb[:, :], 0.0)
            # init: alpha0[0] = e(t=0, blank), alpha0[1] = e(t=0, label0)
            nc.scalar.copy(out=A[:, 2:4], in_=e51c[0][:, 0:2])

            cur, nxt = A, Bb
            k = 0
            pend = False  # renorm recorded last step; fold 1/S this step
            for t in range(1, T):
                j, tf = t // 16, t % 16
                et = e51c[j][:, tf * G:tf * G + S]
                emt = e51c[j][:, tf * G + S + 1:tf * G + 2 * S + 1]
                # nxt[s] = (cur[s] + cur[s-1])*e_t[s] + cur[s-2]*em_t[s]
                nc.vector.tensor_add(out=u[:, :], in0=cur[:, 2:S + 2],
                                     in1=cur[:, 1:S + 1])
                if pend:
                    nc.vector.scalar_tensor_tensor(
                        out=u[:, :], in0=u[:, :], scalar=rec[:, :],
                        in1=et, op0=MUL, op1=MUL)
                    nc.vector.scalar_tensor_tensor(
                        out=w[:, :], in0=cur[:, 0:S], scalar=rec[:, :],
                        in1=emt, op0=MUL, op1=MUL)
                    pend = False
                else:
                    nc.vector.tensor_mul(out=u[:, :], in0=u[:, :], in1=et)
                    nc.vector.tensor_mul(out=w[:, :], in0=cur[:, 0:S],
                                         in1=emt)
                if t % RENORM == 0 and t <= 152:
                    nc.vector.scalar_tensor_tensor(
                        out=nxt[:, 2:S + 2], in0=u[:, :], scalar=0.0,
                        in1=w[:, :], op0=ADD, op1=ADD,
                        accum_out=Sb[:, k:k + 1])
                    nc.vector.reciprocal(out=rec[:, :], in_=Sb[:, k:k + 1])
                    k += 1
                    pend = True
                else:
                    nc.vector.tensor_add(out=nxt[:, 2:S + 2], in0=u[:, :],
                                         in1=w[:, :])
                cur, nxt = nxt, cur
            assert k == NREN

            # ---- raw readout; host does the scalar math ----
            nc.sync.dma_start(out=alpha_out[:, :], in_=cur[:, :])
            nc.sync.dma_start(out=sb_out[:, :], in_=Sb[:, :])
            nc.sync.dma_start(out=z_out[:, :], in_=Z[:, :])

    nc.compile()
    return nc


def _prep_core(predicts, labels, label_lengths, b0):
    """Host-side shard prep for examples [b0, b0+BL)."""
    # permute rows to (t_block, example, t_fine) so streaming tile i = 2j+o
    # holds examples [8o, 8o+8) x timesteps [16j, 16j+16) as 128 contiguous
    # rows (partition p = b_loc*16 + t_fine); cast to bf16 for the DMA
    xs = np.asarray(predicts[b0:b0 + BL]).astype(ml_dtypes.bfloat16)
    xs = np.ascontiguousarray(
        xs.reshape(BL, TBJ, 16, C).transpose(1, 0, 2, 3).reshape(BL * T, C))
    lab = labels[b0:b0 + BL].astype(np.int64)            # [BL, L]
    # extended-label class ids per state: even s -> blank 0, odd s -> label
    ext = np.zeros((BL, S), np.int64)
    ext[:, 1::2] = lab
    # gather slot table: n in 0..50 -> e[s=n]; n in 52..102 -> em[s=n-52]
    # (class id if the skip transition is allowed, else C = zeroed column)
    idx_all = np.full((BL, NI), C, np.int64)
    idx_all[:, 0:S] = ext
    allow = np.zeros((BL, S), bool)
    allow[:, 3::2] = lab[:, 1:] != lab[:, :-1]
    idx_all[:, S + 1:2 * S + 1] = np.where(allow, ext, C)
    # ap_gather idx tiles: streaming tile i, partition p -> example
    # 8*(i%2) + p//16; idx n lives at (partition group_base + n%16, col n//16)
    i_idx = np.arange(NT)[:, None, None]
    p_idx = np.arange(128)[None, :, None]
    c_idx = np.arange(7)[None, None, :]
    b_of = 8 * (i_idx % 2) + p_idx // 16
    n_of = c_idx * 16 + (p_idx % 16)
    gidx = idx_all[b_of, n_of]                           # [NT, 128, 7]
    gidx = gidx.transpose(1, 0, 2).reshape(128, NT * 7).astype(np.int16)
    return {"x": xs, "gidx": gidx}


_NC_CACHE = []


def kernel(predicts, labels, label_lengths):
    predicts = np.asarray(predicts)
    labels = np.asarray(labels)
    label_lengths = np.asarray(label_lengths)
    if not _NC_CACHE:
        _NC_CACHE.append(_build_kernel())
    nc = _NC_CACHE[0]
    in_maps = [
        _prep_core(predicts, labels, label_lengths, k * BL) for k in range(NCORES)
    ]
    res = bass_utils.run_bass_kernel_spmd(nc, in_maps, core_ids=list(range(NCORES)))

    # host readout: loss = sum_t log Z - (log v + sum_k log S_k), then focal
    losses = np.empty(B, np.float64)
    i_idx = np.arange(NT)[None, :]
    p_idx = np.arange(128)[:, None]
    b_of = 8 * (i_idx % 2) + p_idx // 16                 # [128, NT]
    for kk in range(NCORES):
        r = res.results[kk]
        alphaT = r["alpha_out"].astype(np.float64)       # [BL, S+2]
        sb = r["sb_out"].astype(np.float64)              # [BL, NREN]
        z = r["z_out"].astype(np.float64)                # [128, NT]
        lz = np.log(z)
        sum_lz = np.zeros(BL)
        np.add.at(sum_lz, b_of.ravel(), lz.ravel())
        lens = label_lengths[kk * BL:(kk + 1) * BL].astype(np.int64)
        rows = np.arange(BL)
        v = alphaT[rows, 2 * lens + 2] + alphaT[rows, 2 * lens + 1]
        losses[kk * BL:(kk + 1) * BL] = (
            sum_lz - (np.log(v) + np.sum(np.log(sb), axis=1)))
    weight = np.square(1.0 - np.exp(-np.minimum(losses, 80.0)))
    return np.float32(np.mean(losses * weight))


# revision 4
# speedup vs baseline: 1.6041x; 1.0628x over previous
"""CTC loss (focal-reweighted) Trainium2 Bass kernel.

Strategy: pure data parallel over batch (128 examples -> 8 cores x 16).
Per core:
  - stream x tiles of [8 examples x 16 timesteps, C] in bf16 (host-cast;
    halves the dominant HBM traffic); exp on ACT (bf16 in -> f32 out tile)
    with accum_out -> softmax denominators Z[b,t]
  - ap_gather (GPSIMD) pulls per-(b,t) emission values out of the exp'd
    tile in extended-label order TWICE: slots 0..50 are e[s], slots
    52..102 are em[s] = e[s]*allow_skip[s] (mask folded into the gather by
    pointing disallowed slots at a zeroed column past C) - this removes
    the mask multiply from the DP chain
  - gathered values reshuffle SBUF->SBUF into per-t-block chunks (GPSIMD
    SWDGE queue) so the CTC forward DP pipelines behind the streaming
  - DP in rescaled prob space: 4 DVE ops/step; renorm every 8 steps is
    folded in (sum via scalar_tensor_tensor accum_out on the step's final
    add, 1/S applied by scalar_tensor_tensor on the NEXT step's two
    product terms) so it only costs one reciprocal on the serial chain
  - outputs alpha_T, the 19 renorm sums and Z raw; host does all the
    log/focal/mean scalar math in float64
"""

import numpy as np
import ml_dtypes

import concourse.bass as bass
import concourse.bacc as bacc
import concourse.tile as tile
from concourse import mybir
from concourse import bass_utils

B, T, C, L = 128, 160, 6625, 25
NCORES = 8
BL = B // NCORES          # 16 examples per core
S = 2 * L + 1             # 51 extended states
NI = 112                  # ap_gather num_idxs (7 idx cols x 16 partitions)
G = 104                   # chunk stride: e at 0..50, em at 52..102
TBJ = 10                  # t-blocks of 16 timesteps
NT = 2 * TBJ              # 20 streaming tiles of [128, C]
CP = C + 8                # exp'd tile width; cols C.. stay 0 (em mask)
RENORM = 8
NREN = 19                 # renorms at t = 8,16,...,152
NXIN = 4                  # bf16 input tile ring
NXOUT = 3                 # f32 exp'd tile ring
NGA = 4                   # gather output ring

F32 = mybir.dt.float32
BF16 = mybir.dt.bfloat16
I16 = mybir.dt.int16


def _build_kernel():
    nc = bacc.Bacc("TRN2", target_bir_lowering=False, debug=False)
    x = nc.dram_tensor("x", [BL * T, C], BF16, kind="ExternalInput").ap()
    gidx = nc.dram_tensor("gidx", [128, NT * 7], I16, kind="ExternalInput").ap()
    alpha_out = nc.dram_tensor("alpha_out", [BL, S + 2], BF16,
                               kind="ExternalOutput").ap()
    sb_out = nc.dram_tensor("sb_out", [BL, NREN], F32,
                            kind="ExternalOutput").ap()
    z_out = nc.dram_tensor("z_out", [128, NT], F32, kind="ExternalOutput").ap()

    MUL = mybir.AluOpType.mult
    ADD = mybir.AluOpType.add

    with tile.TileContext(nc) as tc:
        with tc.tile_pool(name="small", bufs=1) as small:
            gidx_sb = small.tile([128, NT * 7], I16)
            nc.scalar.dma_start(out=gidx_sb[:, :], in_=gidx[:, :])

            xin = [small.tile([128, C], BF16, name=f"xin{n}")
                   for n in range(NXIN)]
            xout = [small.tile([128, CP], F32, name=f"xout{n}")
                    for n in range(NXOUT)]
            ga = [small.tile([128, NI], F32, name=f"ga{n}") for n in range(NGA)]
            for n in range(NXOUT):
                # zero guard columns: em-gather indices for masked slots
                # point at col C, so these must stay 0 (exp only writes :C)
                nc.vector.memset(xout[n][:, C:CP], 0.0)

            Z = small.tile([128, NT], F32)
            e51c = [small.tile([BL, 16 * G], BF16, name=f"e51c{j}")
                    for j in range(TBJ)]

            # ---- streaming: tile i = 2j+o holds examples [8o, 8o+8) x
            # timesteps [16j, 16j+16); partition p = b_loc*16 + t_fine ----
            xv = x.rearrange("(n p) c -> n p c", p=128)
            for i in range(NT):
                xt = xin[i % NXIN]
                nc.sync.dma_start(out=xt[:, :], in_=xv[i, :, :])
                xo = xout[i % NXOUT]
                nc.scalar.activation(out=xo[:, 0:C], in_=xt[:, :],
                                     func=mybir.ActivationFunctionType.Exp,
                                     accum_out=Z[:, i:i + 1])
                g = ga[i % NGA]
                nc.gpsimd.ap_gather(
                    out_ap=g[:, :].rearrange("p (n d) -> p n d", d=1),
                    in_ap=xo[:, :].rearrange("p (n d) -> p n d", d=1),
                    idxs_ap=gidx_sb[:, i * 7:(i + 1) * 7],
                    channels=128, num_elems=CP, d=1, num_idxs=NI,
                )
                # SBUF->SBUF partition reshuffle straight into the DP chunk
                # (SWDGE so it doesn't FIFO behind x-loads on the sync ring)
                j, o = i // 2, i % 2
                ecv = e51c[j][:, :].rearrange("b (t g) -> b t g", g=G)
                nc.gpsimd.dma_start(out=ecv[8 * o:8 * o + 8, :, :],
                                    in_=g[:, 0:G])

            # ---- CTC forward DP in rescaled prob space ----
            # alpha buffers have 2 guard columns (always 0); state s at
            # col s+2, so cur[:, 0:S] reads alpha[s-2] (guards give 0)
            A = small.tile([BL, S + 2], BF16)
            Bb = small.tile([BL, S + 2], BF16)
            u = small.tile([BL, S], BF16)
            w = small.tile([BL, S], BF16)
            Sb = small.tile([BL, NREN], F32)
            rec = small.tile([BL, 1], F32)
            nc.vector.memset(A[:, :], 0.0)
            nc.vector.memset(Bb[:, :], 0.0)
            # init: alpha0[0] = e(t=0, blank), alpha0[1] = e(t=0, label0)
            nc.scalar.copy(out=A[:, 2:4], in_=e51c[0][:, 0:2])

            cur, nxt = A, Bb
            k = 0
            pend = False  # renorm recorded last step; fold 1/S this step
            for t in range(1, T):
                j, tf = t // 16, t % 16
                et = e51c[j][:, tf * G:tf * G + S]
                emt = e51c[j][:, tf * G + S + 1:tf * G + 2 * S + 1]
                # nxt[s] = (cur[s] + cur[s-1])*e_t[s] + cur[s-2]*em_t[s]
                nc.vector.tensor_add(out=u[:, :], in0=cur[:, 2:S + 2],
                                     in1=cur[:, 1:S + 1])
                if pend:
                    nc.vector.scalar_tensor_tensor(
                        out=u[:, :], in0=u[:, :], scalar=rec[:, :],
                        in1=et, op0=MUL, op1=MUL)
                    nc.vector.scalar_tensor_tensor(
                        out=w[:, :], in0=cur[:, 0:S], scalar=rec[:, :],
                        in1=emt, op0=MUL, op1=MUL)
                    pend = False
                else:
                    nc.vector.tensor_mul(out=u[:, :], in0=u[:, :], in1=et)
                    nc.vector.tensor_mul(out=w[:, :], in0=cur[:, 0:S],
                                         in1=emt)
                if t % RENORM == 0 and t <= 152:
                    nc.vector.scalar_tensor_tensor(
                        out=nxt[:, 2:S + 2], in0=u[:, :], scalar=0.0,
                        in1=w[:, :], op0=ADD, op1=ADD,
                        accum_out=Sb[:, k:k + 1])
                    nc.vector.reciprocal(out=rec[:, :], in_=Sb[:, k:k + 1])
                    k += 1
                    pend = True
                else:
                    nc.vector.tensor_add(out=nxt[:, 2:S + 2], in0=u[:, :],
                                         in1=w[:, :])
                cur, nxt = nxt, cur
            assert k == NREN

            # ---- raw readout; host does the scalar math ----
            nc.sync.dma_start(out=alpha_out[:, :], in_=cur[:, :])
            nc.sync.dma_start(out=sb_out[:, :], in_=Sb[:, :])
            nc.sync.dma_start(out=z_out[:, :], in_=Z[:, :])

    nc.compile()
    return nc


def _prep_core(predicts, labels, label_lengths, b0):
    """Host-side shard prep for examples [b0, b0+BL)."""
    # permute rows to (t_block, example, t_fine) so streaming tile i = 2j+o
    # holds examples [8o, 8o+8) x timesteps [16j, 16j+16) as 128 contiguous
    # rows (partition p = b_loc*16 + t_fine); cast to bf16 for the DMA
    xs = np.asarray(predicts[b0:b0 + BL]).astype(ml_dtypes.bfloat16)
    xs = np.ascontiguousarray(
        xs.reshape(BL, TBJ, 16, C).transpose(1, 0, 2, 3).reshape(BL * T, C))
    lab = labels[b0:b0 + BL].astype(np.int64)            # [BL, L]
    # extended-label class ids per state: even s -> blank 0, odd s -> label
    ext = np.zeros((BL, S), np.int64)
    ext[:, 1::2] = lab
    # gather slot table: n in 0..50 -> e[s=n]; n in 52..102 -> em[s=n-52]
    # (class id if the skip transition is allowed, else C = zeroed column)
    idx_all = np.full((BL, NI), C, np.int64)
    idx_all[:, 0:S] = ext
    allow = np.zeros((BL, S), bool)
    allow[:, 3::2] = lab[:, 1:] != lab[:, :-1]
    idx_all[:, S + 1:2 * S + 1] = np.where(allow, ext, C)
    # ap_gather idx tiles: streaming tile i, partition p -> example
    # 8*(i%2) + p//16; idx n lives at (partition group_base + n%16, col n//16)
    i_idx = np.arange(NT)[:, None, None]
    p_idx = np.arange(128)[None, :, None]
    c_idx = np.arange(7)[None, None, :]
    b_of = 8 * (i_idx % 2) + p_idx // 16
    n_of = c_idx * 16 + (p_idx % 16)
    gidx = idx_all[b_of, n_of]                           # [NT, 128, 7]
    gidx = gidx.transpose(1, 0, 2).reshape(128, NT * 7).astype(np.int16)
    return {"x": xs, "gidx": gidx}


_NC_CACHE = []


def kernel(predicts, labels, label_lengths):
    predicts = np.asarray(predicts)
    labels = np.asarray(labels)
    label_lengths = np.asarray(label_lengths)
    if not _NC_CACHE:
        _NC_CACHE.append(_build_kernel())
    nc = _NC_CACHE[0]
    in_maps = [
        _prep_core(predicts, labels, label_lengths, k * BL) for k in range(NCORES)
    ]
    res = bass_utils.run_bass_kernel_spmd(nc, in_maps, core_ids=list(range(NCORES)))

    # host readout: loss = sum_t log Z - (log v + sum_k log S_k), then focal
    losses = np.empty(B, np.float64)
    i_idx = np.arange(NT)[None, :]
    p_idx = np.arange(128)[:, None]
    b_of = 8 * (i_idx % 2) + p_idx // 16                 # [128, NT]
    for kk in range(NCORES):
        r = res.results[kk]
        alphaT = np.asarray(r["alpha_out"]).astype(np.float64)       # [BL, S+2]
        sb = r["sb_out"].astype(np.float64)              # [BL, NREN]
        z = r["z_out"].astype(np.float64)                # [128, NT]
        lz = np.log(z)
        sum_lz = np.zeros(BL)
        np.add.at(sum_lz, b_of.ravel(), lz.ravel())
        lens = label_lengths[kk * BL:(kk + 1) * BL].astype(np.int64)
        rows = np.arange(BL)
        v = alphaT[rows, 2 * lens + 2] + alphaT[rows, 2 * lens + 1]
        losses[kk * BL:(kk + 1) * BL] = (
            sum_lz - (np.log(v) + np.sum(np.log(sb), axis=1)))
    weight = np.square(1.0 - np.exp(-np.minimum(losses, 80.0)))
    return np.float32(np.mean(losses * weight))
